# revision 7
# baseline (speedup 1.0000x reference)
"""Trainium2 Bass kernel for nn_MultiHeadAttention_84318797955257.

Inverted-softmax multi-head attention, 8-core SPMD.

  - Sharding: 4 batches x 2 head-halves (each core: 1 batch, all 1024 query
    rows, 8 of 16 heads, full S).  Wq/Wk/Wv are column-sliced and Wo
    row-sliced per head-half, so no projection work is duplicated anywhere
    (the previous batch x T-half layout duplicated the K/V projections on
    both cores of a pair).  The row-parallel Wo partial sums are added
    during the host-side unshard (one f16 partial per core, pair-summed in
    f32); the only device collective is the tiny inverted-softmax
    denominator Z[b,s]: a [128,4] f32 AllReduce per s-half over core pairs.
  - Mask folded into bias host-side as -240 additive fp8_e4m3 (exp
    underflows to 0, matching the reference's where(mask,0)); DH**-0.5
    folded into Wq.  Bias streams as fp8 (8.4MB/core, half the f16 cost).
  - Scores are computed in [s, t] layout (eT = khT.T @ qhT per head) so the
    AV matmul needs no transpose.  Per-head K=64 score matmuls are packed
    pairwise into disjoint PE row-groups and run concurrently.  The bias
    lands via DVE tensor_add (PSUM f32 + fp8 -> f16 staging); this frees
    the PE from the 128 identity-matmul bias adds the old kernel used
    (~28us of PE), and exp then runs on ACT from SBUF in N=2048 chunks
    (2 heads x 1024 t) with accum_out producing the Z partial sums.
  - ~20 dummy matmuls on a zero scratch tile at kernel start keep the PE
    busy while the first DMAs land, flipping the HAM clock-gate to 8/8
    before real work starts; a short dummy-matmul bridge between D1 and D2
    keeps it warm across the second AllReduce wait.
  - 1/Z is folded into V rows (v' = v/Z); an extra ones-column in V yields
    the S-renorm denominator r as row 64 of the AV output.  AV runs in two
    waves: D1 (s-tiles 0-3, after AR0) is emitted behind the B1 score loop
    as PE filler; D2 (s-tiles 4-7, after AR1) merges the D1 partial via a
    DVE add.  r-rows are collected cross-partition into a
    [{0,32,64,96} x 2] layout so one Ln + one Exp per 4-head group computes
    1/(r+eps), then col-packed K=1 outer-product matmuls broadcast it
    across partitions for the final numerator scaling.
  - All host-side input arrays are pre-tiled so every DMA descriptor is a
    contiguous >=1KB per-partition line.
"""

import numpy as np
import ml_dtypes
import bass_rust
import concourse.bass as bass
import concourse.mybir as mybir
import concourse.tile as tile
from concourse.bass_utils import run_bass_kernel_spmd
from concourse.masks import make_identity

AF = mybir.ActivationFunctionType
ALU = mybir.AluOpType
f32 = mybir.dt.float32
f16 = mybir.dt.float16
fp8 = mybir.dt.float8e4

B, T, S, D, H, DH = 4, 1024, 1024, 1024, 16, 64
P = 128
HL = 8            # heads per core
FL = HL * DH      # 512 local features
NEG = -240.0      # mask additive, exactly representable in fp8_e4m3
EPS = 1e-5
NCORES = 8
REPLICA_GROUPS = [[0, 1], [2, 3], [4, 5], [6, 7]]
NWARM = 14        # startup HAM-warming dummy matmuls
NBRIDGE = 24      # AR1-window dummy matmuls


# ---------------------------------------------------------------------------
# Workaround: this container's walrus build allows only ONE sync-wait per
# instruction ("Too many sync wait commands" in setupSyncWait).  After Tile
# scheduling, split any instruction's extra waits onto preceding NOPs on the
# same engine (the engine sequencer blocks on each wait in program order, so
# semantics are identical).
# ---------------------------------------------------------------------------
def _split_multi_waits(nc):
    cnt = 0
    for fn in nc.m.functions:
        for bb in fn.blocks:
            new = []
            changed = False
            for inst in bb.instructions:
                si = inst.sync_info
                if si is not None and len(si.on_wait) > 1:
                    changed = True
                    waits = list(si.on_wait)
                    for w in waits[:-1]:
                        cnt += 1
                        nop = mybir.InstNoOp(
                            name=f"I-waitsplit-{cnt}", ins=[], outs=[]
                        )
                        nop.engine = inst.engine
                        nop.sync_info = bass_rust.SyncInfo(
                            on_wait=[w], on_update=[]
                        )
                        new.append(nop)
                    inst.sync_info = bass_rust.SyncInfo(
                        on_wait=[waits[-1]], on_update=list(si.on_update)
                    )
                new.append(inst)
            if changed:
                bb.instructions = new
    return cnt


# ---------------------------------------------------------------------------
# Device program (identical on all 8 cores)
# ---------------------------------------------------------------------------
def _emit(nc, tc, qtl, ktl, vtl, wql, wkl, wvl, wol, biasl, out):
    from contextlib import ExitStack

    with ExitStack() as ctx:
        perst = ctx.enter_context(tc.tile_pool(name="perst", bufs=1))
        scratch = perst.tile([P, 512], f16)
        nc.vector.memset(scratch[:], 0.0)
        ones_bc = perst.tile([P, 64], f16)
        nc.gpsimd.memset(ones_bc[:], 1.0)
        ones_col = perst.tile([P, 8], f32)
        nc.gpsimd.memset(ones_col[:], 1.0)
        eps_t = perst.tile([P, 1], f32)
        nc.gpsimd.memset(eps_t[:], EPS)
        ident8 = perst.tile([P, P], fp8)
        make_identity(nc, ident8[:])
        ident16 = perst.tile([P, P], f16)
        make_identity(nc, ident16[:])
        zball = perst.tile([P, 64], f32)   # exp accum per (i, f, th)
        nc.gpsimd.memset(zball[:], 0.0)
        zloc = perst.tile([P, 8], f32)
        zsink = perst.tile([P, 8], f32)
        zg = perst.tile([P, 8], f32)
        zinv = perst.tile([P, 8], f32)

        vaug_pool = ctx.enter_context(tc.tile_pool(name="vaugp", bufs=1))
        vaug = [vaug_pool.tile([P, HL, 65], f16, name=f"vaug{i}")
                for i in range(8)]
        drp = ctx.enter_context(tc.tile_pool(name="drp", bufs=1, space="DRAM"))
        din = [drp.tile([P, 4], f32, name=f"din{s}") for s in range(2)]
        dout = [drp.tile([P, 4], f32, name=f"dout{s}") for s in range(2)]
        ddin = drp.tile([P, 4], f32, name="ddin")
        ddout = drp.tile([P, 4], f32, name="ddout")
        # dummy AllReduce: the first CC trigger pays ~11.5us of one-time
        # stream-init cost; swallow it during phase A so the real ARs are
        # prompt.
        nc.gpsimd.dma_start(ddin[:], ones_col[:, 0:4])
        nc.gpsimd.collective_compute(
            "AllReduce", ALU.add, replica_groups=REPLICA_GROUPS,
            ins=[ddin.opt()], outs=[ddout.opt()],
        )

        def _ar_trigger(sh):
            nc.gpsimd.dma_start(din[sh][:], zloc[:, sh * 4:(sh + 1) * 4])
            nc.gpsimd.collective_compute(
                "AllReduce", ALU.add, replica_groups=REPLICA_GROUPS,
                ins=[din[sh].opt()], outs=[dout[sh].opt()],
            )

        def _ar_consume(sh):
            nc.gpsimd.dma_start(zg[:, sh * 4:(sh + 1) * 4], dout[sh][:])
            nc.vector.reciprocal(zinv[:, sh * 4:(sh + 1) * 4],
                                 zg[:, sh * 4:(sh + 1) * 4])
            for i in range(sh * 4, sh * 4 + 4):
                nc.vector.tensor_scalar_mul(
                    vaug[i][:, :, 0:64], vaug[i][:, :, 0:64],
                    zinv[:, i:i + 1],
                )
                nc.vector.tensor_scalar_mul(
                    vaug[i][:, :, 64:65], ones_col[:, :, None],
                    zinv[:, i:i + 1],
                )

        # qh/kh live from phase A through the end of B1 scores; the pool
        # stays open (stack allocator) but is small.
        qk_pool = ctx.enter_context(tc.tile_pool(name="qk", bufs=1))
        qh = qk_pool.tile([P, 4, T], f16)      # qhT: [feat_loc, f_tile, t]
        kh = qk_pool.tile([P, 4, S], f16)      # khT: [feat_loc, f_tile, s]

        # ---- phase A: q/k projections, with HAM warmup while DMAs land ----
        with ExitStack() as actx:
            apool = actx.enter_context(tc.tile_pool(name="apool", bufs=1))
            atp = actx.enter_context(tc.tile_pool(name="atp", bufs=1))
            apsum = actx.enter_context(
                tc.tile_pool(name="apsum", bufs=2, space="PSUM")
            )
            wps = apsum.tile([P, 512], f32, tag="warm")
            for _ in range(NWARM):
                nc.tensor.matmul(
                    wps[:], scratch[:, 0:128], scratch[:],
                    start=True, stop=True,
                )
            qt = atp.tile([P, 8, T], f16)
            kt = atp.tile([P, 8, S], f16)
            wq = [apool.tile([P, 8, P], f16, name=f"wq{f}") for f in range(4)]
            wk = [apool.tile([P, 8, P], f16, name=f"wk{f}") for f in range(4)]
            # first matmul needs wq0 + qt[:, 0:2] only; stream the rest behind
            nc.sync.dma_start(wq[0][:], wql[:, 0])
            nc.sync.dma_start(qt[:, 0:2, :], qtl[:, 0:2, :])
            nc.sync.dma_start(qt[:, 2:4, :], qtl[:, 2:4, :])
            nc.sync.dma_start(wq[1][:], wql[:, 1])
            nc.sync.dma_start(qt[:, 4:6, :], qtl[:, 4:6, :])
            nc.sync.dma_start(qt[:, 6:8, :], qtl[:, 6:8, :])
            nc.sync.dma_start(wq[2][:], wql[:, 2])
            nc.sync.dma_start(wq[3][:], wql[:, 3])

            for f in range(4):
                # kt + wk stream in behind the Q-projection matmuls
                nc.sync.dma_start(kt[:, 2 * f:2 * f + 2, :],
                                  ktl[:, 2 * f:2 * f + 2, :])
                nc.sync.dma_start(wk[f][:], wkl[:, f])
                ps = apsum.tile([P, 2, 512], f32, tag="qk")
                for c in range(8):
                    lw = wq[f][:, c, :]
                    nc.tensor.matmul(
                        ps[:, 0, :], lw, qt[:, c, 0:512],
                        start=(c == 0), stop=(c == 7),
                    )
                    nc.tensor.matmul(
                        ps[:, 1, :], lw, qt[:, c, 512:1024],
                        start=(c == 0), stop=(c == 7),
                    )
                # DH**-0.5 folded into wql host-side; evac on ACT (idle here)
                nc.scalar.copy(
                    qh[:, f, :].rearrange("p (b t) -> p b t", b=2), ps[:]
                )
            for f in range(4):
                ps = apsum.tile([P, 2, 512], f32, tag="qk")
                for c in range(8):
                    lw = wk[f][:, c, :]
                    nc.tensor.matmul(
                        ps[:, 0, :], lw, kt[:, c, 0:512],
                        start=(c == 0), stop=(c == 7),
                    )
                    nc.tensor.matmul(
                        ps[:, 1, :], lw, kt[:, c, 512:1024],
                        start=(c == 0), stop=(c == 7),
                    )
                nc.scalar.copy(
                    kh[:, f, :].rearrange("p (b s) -> p b s", b=2), ps[:]
                )

        # exp-weight buffer: opened after A so its 128KB/partition reuses
        # the qt/kt space; D1 accumulators + Wo likewise live here.
        wbuf_pool = ctx.enter_context(tc.tile_pool(name="wbufp", bufs=1))
        wre = wbuf_pool.tile([P, 8, HL, T], f16)
        d1pool = ctx.enter_context(tc.tile_pool(name="d1pool", bufs=1))
        tmpA = [d1pool.tile([P, 2, 512], f16, name=f"tmpA{h}")
                for h in range(HL)]
        # B-phase streaming pools (closed right after B to free their space
        # for the D/E pools -- opened last so the stack pops cleanly)
        bstuff = ctx.enter_context(ExitStack())
        ebp = bstuff.enter_context(tc.tile_pool(name="ebp", bufs=3))
        ebt = {}
        for i in range(8):
            for f in range(4):
                ebt[(i, f)] = ebp.tile(
                    [P, 4, 512], fp8, tag="eb", name=f"eb{i}_{f}"
                )
        stgp = bstuff.enter_context(tc.tile_pool(name="stgp", bufs=2))
        vpool = bstuff.enter_context(tc.tile_pool(name="vpool", bufs=1))
        wvpool = bstuff.enter_context(tc.tile_pool(name="wvpool", bufs=1))
        wv = wvpool.tile([P, 8, FL], f16)
        vt0 = vpool.tile([P, 8, 512], f16, tag="vt", name="vt0")
        # B0-front prefetches (the DMA queue drains these behind phase A's
        # transfers, overlapping the A matmuls)
        nc.sync.dma_start(ebt[(0, 0)][:], biasl[0, 0])
        nc.sync.dma_start(ebt[(0, 1)][:], biasl[0, 1])
        nc.gpsimd.dma_start(vt0[:, 0:4, :], vtl[:, 0:4, 0:512])
        nc.gpsimd.dma_start(vt0[:, 4:8, :], vtl[:, 4:8, 0:512])
        nc.gpsimd.dma_start(wv[:, 0:4, :], wvl[:, 0:4, :])
        nc.gpsimd.dma_start(wv[:, 4:8, :], wvl[:, 4:8, :])

        # ---- phases B+C chunked by s-half, V-projection matmuls emitted as
        # PE filler inside the B0 loop, with a split Z-allreduce. ----
        # Bias-add alternates between two paths per (i, f) group so neither
        # engine paces the phase: even f -> PE identity-matmul adds (exp
        # straight from PSUM); odd f -> DVE tensor_add into an f16 staging
        # tile (exp from SBUF, N=2048).
        def _score_mms(ps, i, f, th, stop):
            for u in range(2):
                po = u * 64
                nc.tensor.matmul(
                    ps[:, u, :],
                    kh[po:po + 64, f, i * P:(i + 1) * P],
                    qh[po:po + 64, f, th * 512:(th + 1) * 512],
                    start=True, stop=stop,
                )

        def _unit_pe(bpsum, i, f):
            zi = (i * 4 + f) * 2
            for th in range(2):
                ps = bpsum.tile([P, 2, 512], f32, tag="eps")
                _score_mms(ps, i, f, th, stop=False)
                for u in range(2):
                    nc.tensor.matmul(
                        ps[:, u, :], ident8[:], bt_cur[0][:, th * 2 + u, :],
                        start=False, stop=True,
                    )
                nc.scalar.activation(
                    wre[:, i, 2 * f:2 * f + 2, th * 512:(th + 1) * 512],
                    ps[:], AF.Exp, accum_out=zball[:, zi + th:zi + th + 1],
                )

        def _unit_dve(bpsum, i, f):
            zi = (i * 4 + f) * 2
            stg_t = stgp.tile([P, 2, 2, 512], f16, tag="stg")
            for th in range(2):
                ps = bpsum.tile([P, 2, 512], f32, tag="eps")
                _score_mms(ps, i, f, th, stop=True)
                nc.vector.tensor_add(
                    stg_t[:, :, th, :], ps[:], bt_cur[0][:, th * 2:th * 2 + 2, :]
                )
            nc.scalar.activation(
                wre[:, i, 2 * f:2 * f + 2, :],
                stg_t[:].rearrange("p u a b -> p u (a b)"),
                AF.Exp, accum_out=zball[:, zi:zi + 1],
            )

        bt_cur = [None]

        def _c_chunk(cp, il, vt, crange):
            for c in crange:
                nc.tensor.matmul(
                    cp[:], vt[:, c, il * P:(il + 1) * P], wv[:, c, :],
                    start=(c == 0), stop=(c == 7),
                )

        def _c_evac(cp, i):
            nc.vector.tensor_copy(
                vaug[i][:, :, 0:64],
                cp[:].rearrange("p (h c) -> p h c", c=64),
            )

        with ExitStack() as bctx:
            bpsum = bctx.enter_context(
                tc.tile_pool(name="bpsum", bufs=2, space="PSUM")
            )
            cpsum = bctx.enter_context(
                tc.tile_pool(name="cpsum", bufs=1, space="PSUM")
            )
            dpsum = bctx.enter_context(
                tc.tile_pool(name="dpsum", bufs=1, space="PSUM")
            )
            # ---- B0 (+ interleaved C0) ----
            for i in range(4):
                cp = cpsum.tile([P, 512], f32, tag="vps", name=f"vps{i}")
                for f in range(4):
                    bt_cur[0] = ebt[(i, f)]
                    if not (i == 0 and f <= 1):
                        nc.sync.dma_start(bt_cur[0][:], biasl[i, f])
                    if f % 2 == 0:
                        _unit_pe(bpsum, i, f)
                    else:
                        _unit_dve(bpsum, i, f)
                    _c_chunk(cp, i, vt0, (2 * f, 2 * f + 1))
                    if f == 3:
                        _c_evac(cp, i)
                nc.scalar.activation(
                    zsink[:], zball[:, i * 8:(i + 1) * 8], AF.Copy,
                    accum_out=zloc[:, i:i + 1],
                )
            _ar_trigger(0)

            # ---- B1: no C filler (held back for the AR1 window) ----
            vt1 = vpool.tile([P, 8, 512], f16, tag="vt", name="vt1")
            nc.gpsimd.dma_start(vt1[:, 0:4, :], vtl[:, 0:4, 512:1024])
            nc.gpsimd.dma_start(vt1[:, 4:8, :], vtl[:, 4:8, 512:1024])
            for i in range(4, 8):
                for f in range(4):
                    bt_cur[0] = ebt[(i, f)]
                    nc.sync.dma_start(bt_cur[0][:], biasl[i, f])
                    if f % 2 == 0:
                        _unit_pe(bpsum, i, f)
                    else:
                        _unit_dve(bpsum, i, f)
                nc.scalar.activation(
                    zsink[:], zball[:, i * 8:(i + 1) * 8], AF.Copy,
                    accum_out=zloc[:, i:i + 1],
                )
            _ar_trigger(1)
            _ar_consume(0)

            # held-back s-half-1 v-projection covers the AR1 window
            for il in range(4):
                i = 4 + il
                cp = cpsum.tile([P, 512], f32, tag="vps", name=f"vps{i}")
                _c_chunk(cp, il, vt1, range(8))
                _c_evac(cp, i)

            # D1: s-tiles 0-3 AV chains for ALL heads (independent of the
            # second Z allreduce) -- PE filler behind B1 in the queue.  The
            # two t-halves chain into single-bank tiles from two pools so
            # consecutive heads pipeline without waiting on the ACT evac.
            for h in range(HL):
                psA0 = dpsum.tile([P, 512], f32, tag="avp")
                psA1 = cpsum.tile([P, 512], f32, tag="vps")
                for i in range(4):
                    lw = vaug[i][:, h, :]
                    nc.tensor.matmul(
                        psA0[0:65, :], lw, wre[:, i, h, 0:512],
                        start=(i == 0), stop=(i == 3),
                    )
                    nc.tensor.matmul(
                        psA1[0:65, :], lw, wre[:, i, h, 512:1024],
                        start=(i == 0), stop=(i == 3),
                    )
                nc.scalar.copy(tmpA[h][0:65, 0, :], psA0[0:65, :])
                nc.scalar.copy(tmpA[h][0:65, 1, :], psA1[0:65, :])

            _ar_consume(1)

            # dummy-matmul bridge keeps HAM warm across the AR1 wait
            for _ in range(NBRIDGE):
                br = dpsum.tile([P, 512], f32, tag="avp")
                nc.tensor.matmul(
                    br[:], scratch[:, 0:128], scratch[:],
                    start=True, stop=True,
                )

        bstuff.close()   # ebt/stg/vt/wv space freed for the D/E pools

        # ---- phase D2: AV s-tiles 4-7 + grouped renorm ----
        dstack = ctx.enter_context(ExitStack())
        dpool = dstack.enter_context(tc.tile_pool(name="dpool", bufs=1))
        wo = dpool.tile([P, 4, D], f16)
        for ch in range(4):
            nc.sync.dma_start(wo[:, ch, :], wol[:, ch])
        # r-rows for all 8 heads: head h -> partition 32*(h%4), block h//4.
        # memset to 1 so Ln on unused rows is finite.
        rcol = dpool.tile([P, 2, T], f32)
        nc.gpsimd.memset(rcol[:], 1.0)
        rinvc = dpool.tile([P, 2, T], f16)
        aop = [dpool.tile([P, T], f16, name=f"aop{j}") for j in range(4)]

        d2stack = ctx.enter_context(ExitStack())
        d2psum = d2stack.enter_context(
            tc.tile_pool(name="d2psum", bufs=2, space="PSUM")
        )
        pbp = d2stack.enter_context(
            tc.tile_pool(name="pbp", bufs=1, space="PSUM")
        )
        for h in range(HL):
            psB = d2psum.tile([P, 2, 512], f32, tag="avp2")
            for i in range(4, 8):
                lw = vaug[i][:, h, :]
                nc.tensor.matmul(
                    psB[0:65, 0, :], lw, wre[:, i, h, 0:512],
                    start=(i == 4), stop=False,
                )
                nc.tensor.matmul(
                    psB[0:65, 1, :], lw, wre[:, i, h, 512:1024],
                    start=(i == 4), stop=False,
                )
            # merge the D1 partial via an identity matmul (PE adds are
            # cheap; DVE is the scarcer engine here), re-evacuate the summed
            # numerator on ACT, then scatter the r-row into rcol's
            # 32-aligned layout
            for th in range(2):
                nc.tensor.matmul(
                    psB[0:65, th, :], ident16[0:65, 0:65],
                    tmpA[h][0:65, th, :], start=False, stop=True,
                )
            nc.scalar.copy(tmpA[h][0:65, :, :], psB[0:65, :, :])
            rp = 32 * (h % 4)
            nc.vector.tensor_copy(
                rcol[rp:rp + 1, h // 4, :],
                tmpA[h][64:65, :, :].rearrange("p a b -> p (a b)"),
            )
            if h % 4 == 3:
                g = h // 4
                # 1/(r+eps) for this 4-head group in two ACT calls
                nc.scalar.activation(
                    rcol[:, g, :], rcol[:, g, :], AF.Ln, bias=eps_t[:]
                )
                nc.scalar.activation(
                    rinvc[:, g, :], rcol[:, g, :], AF.Exp, scale=-1.0
                )
                for j in (2 * g, 2 * g + 1):
                    pb = pbp.tile([P, 2, 512], f32, tag="bcp")
                    for u in range(2):
                        hh = 2 * j + u
                        rp2 = 32 * (hh % 4)
                        for th in range(2):
                            nc.tensor.matmul(
                                pb[u * 64:(u + 1) * 64, th, :],
                                ones_bc[rp2:rp2 + 1, :],
                                rinvc[rp2:rp2 + 1, g,
                                      th * 512:(th + 1) * 512],
                                start=True, stop=True,
                                tile_position=(rp2, u * 64),
                            )
                    for u in range(2):
                        hh = 2 * j + u
                        nc.vector.tensor_mul(
                            out=aop[j][u * 64:(u + 1) * 64, :],
                            in0=tmpA[hh][0:64, :, :].rearrange(
                                "p a b -> p (a b)"),
                            in1=pb[u * 64:(u + 1) * 64, :, :].rearrange(
                                "p a b -> p (a b)"),
                        )
        d2stack.close()

        # ---- phase E: output projection (row-parallel partial; the pair
        # sum happens in the host-side unshard). ----
        estack = ctx.enter_context(ExitStack())
        epool = estack.enter_context(tc.tile_pool(name="epool", bufs=2))
        opsum = estack.enter_context(
            tc.tile_pool(name="opsum", bufs=2, space="PSUM")
        )
        for tch in range(8):
            pso = opsum.tile([P, 2, 512], f32, tag="outp")
            for j in range(4):
                lw = aop[j][:, tch * P:(tch + 1) * P]
                nc.tensor.matmul(
                    pso[:, 0, :], lw, wo[:, j, 0:512],
                    start=(j == 0), stop=(j == 3),
                )
                nc.tensor.matmul(
                    pso[:, 1, :], lw, wo[:, j, 512:1024],
                    start=(j == 0), stop=(j == 3),
                )
            ot = epool.tile([P, 2, 512], f16, tag="ot")
            nc.scalar.copy(ot[:], pso[:])
            nc.sync.dma_start(
                out[tch * P:(tch + 1) * P, :],
                ot[:].rearrange("p a b -> p (a b)"),
            )


def build_nc():
    nc = bass.Bass(num_devices=NCORES)
    qtl = nc.dram_tensor("qtl", [P, 8, T], f16, kind="ExternalInput")
    ktl = nc.dram_tensor("ktl", [P, 8, S], f16, kind="ExternalInput")
    vtl = nc.dram_tensor("vtl", [P, 8, S], f16, kind="ExternalInput")
    wql = nc.dram_tensor("wql", [P, 4, 8, P], f16, kind="ExternalInput")
    wkl = nc.dram_tensor("wkl", [P, 4, 8, P], f16, kind="ExternalInput")
    wvl = nc.dram_tensor("wvl", [P, 8, FL], f16, kind="ExternalInput")
    wol = nc.dram_tensor("wol", [P, 4, D], f16, kind="ExternalInput")
    biasl = nc.dram_tensor("biasl", [8, 4, P, 4, 512], fp8,
                           kind="ExternalInput")
    out = nc.dram_tensor("out", [T, D], f16, kind="ExternalOutput")
    with tile.TileContext(nc) as tc:
        _emit(nc, tc, qtl, ktl, vtl, wql, wkl, wvl, wol, biasl, out)
    _split_multi_waits(nc)
    return nc


_NC_CACHE = {}


def _get_nc():
    if "nc" not in _NC_CACHE:
        _NC_CACHE["nc"] = build_nc()
    return _NC_CACHE["nc"]


# ---------------------------------------------------------------------------
# Host wrapper
# ---------------------------------------------------------------------------
def _prep_inputs(q, k, v, attn_mask, attn_bias, Wq, Wk, Wv, Wo):
    q = np.asarray(q, np.float32)
    k = np.asarray(k, np.float32)
    v = np.asarray(v, np.float32)
    mask = np.asarray(attn_mask)
    bias = np.asarray(attn_bias, np.float32)
    WqT = np.asarray(Wq, np.float32).T * np.float32(DH ** -0.5)
    WkT = np.ascontiguousarray(np.asarray(Wk, np.float32).T)
    WvT = np.ascontiguousarray(np.asarray(Wv, np.float32).T)
    WoT = np.ascontiguousarray(np.asarray(Wo, np.float32).T)

    def _tile_qkv(x):     # [T, D] -> [p, c, t] f16
        return np.ascontiguousarray(
            x.T.reshape(8, P, T).transpose(1, 0, 2)).astype(np.float16)

    def _tile_wqk(wt, hh):   # [D, D] col-slice -> [p, f, o, g] f16
        sl = wt[:, hh * FL:(hh + 1) * FL]
        return np.ascontiguousarray(
            sl.reshape(8, P, 4, P).transpose(1, 2, 0, 3)).astype(np.float16)

    # bias + additive mask (exp underflows to 0 on device), per head-half,
    # tiled [i, f, p, th*2+u, t'] with h = 2f+u, s = i*128+p, t = th*512+t'
    pre = bias + np.where(mask, np.float32(NEG), np.float32(0.0))[None]
    w_hh = []
    for hh in range(2):
        ph = pre[hh * HL:(hh + 1) * HL]      # [8h, t, s]
        arr = ph.reshape(4, 2, 2, 512, 8, P).transpose(4, 0, 5, 2, 1, 3)
        arr = np.clip(arr, -240.0, 240.0)
        bias8 = np.ascontiguousarray(arr).reshape(8, 4, P, 4, 512) \
            .astype(ml_dtypes.float8_e4m3)
        w_hh.append({
            "wql": _tile_wqk(WqT, hh),
            "wkl": _tile_wqk(WkT, hh),
            "wvl": np.ascontiguousarray(
                WvT[:, hh * FL:(hh + 1) * FL].reshape(8, P, FL)
                .transpose(1, 0, 2)).astype(np.float16),
            "wol": np.ascontiguousarray(
                WoT[hh * FL:(hh + 1) * FL, :].reshape(4, P, D)
                .transpose(1, 0, 2)).astype(np.float16),
            "biasl": bias8,
        })

    in_maps = []
    for c in range(NCORES):
        b, hh = c // 2, c % 2
        if hh == 0:
            qt, kt, vt = _tile_qkv(q[b]), _tile_qkv(k[b]), _tile_qkv(v[b])
        in_maps.append({
            "qtl": qt, "ktl": kt, "vtl": vt, **w_hh[hh],
        })
    return in_maps


def run(inputs, trace=False):
    nc = _get_nc()
    in_maps = _prep_inputs(**inputs)
    res = run_bass_kernel_spmd(
        nc, in_maps, core_ids=list(range(NCORES)), trace=trace,
    )
    full = np.empty((B, T, D), np.float32)
    for b in range(B):
        full[b] = (
            np.asarray(res.results[2 * b]["out"], dtype=np.float32)
            + np.asarray(res.results[2 * b + 1]["out"], dtype=np.float32)
        )
    return full, res


def kernel(**inputs) -> np.ndarray:
    full, _ = run(inputs, trace=False)
    return full


# revision 9
# speedup vs baseline: 1.0672x; 1.0672x over previous
"""Trainium2 Bass kernel for nn_MultiHeadAttention_84318797955257.

Inverted-softmax multi-head attention, 8-core SPMD.

  - Sharding: 4 batches x 2 head-halves (each core: 1 batch, all 1024 query
    rows, 8 of 16 heads, full S).  Wq/Wk/Wv are column-sliced and Wo
    row-sliced per head-half, so no projection work is duplicated anywhere
    (the previous batch x T-half layout duplicated the K/V projections on
    both cores of a pair).  The row-parallel Wo partial sums are added
    during the host-side unshard (one f16 partial per core, pair-summed in
    f32); the only device collective is the tiny inverted-softmax
    denominator Z[b,s]: a [128,4] f32 AllReduce per s-half over core pairs.
  - Mask folded into bias host-side as -240 additive fp8_e4m3 (exp
    underflows to 0, matching the reference's where(mask,0)); DH**-0.5
    folded into Wq.  Bias streams as fp8 (8.4MB/core, half the f16 cost).
  - Scores are computed in [s, t] layout (eT = khT.T @ qhT per head) so the
    AV matmul needs no transpose.  Per-head K=64 score matmuls are packed
    pairwise into disjoint PE row-groups and run concurrently.  The bias
    lands via DVE tensor_add (PSUM f32 + fp8 -> f16 staging); this frees
    the PE from the 128 identity-matmul bias adds the old kernel used
    (~28us of PE), and exp then runs on ACT from SBUF in N=2048 chunks
    (2 heads x 1024 t) with accum_out producing the Z partial sums.
  - ~20 dummy matmuls on a zero scratch tile at kernel start keep the PE
    busy while the first DMAs land, flipping the HAM clock-gate to 8/8
    before real work starts; a short dummy-matmul bridge between D1 and D2
    keeps it warm across the second AllReduce wait.
  - 1/Z is folded into V rows (v' = v/Z); an extra ones-column in V yields
    the S-renorm denominator r as row 64 of the AV output.  AV runs in two
    waves: D1 (s-tiles 0-3, after AR0) is emitted behind the B1 score loop
    as PE filler; D2 (s-tiles 4-7, after AR1) merges the D1 partial via a
    DVE add.  r-rows are collected cross-partition into a
    [{0,32,64,96} x 2] layout so one Ln + one Exp per 4-head group computes
    1/(r+eps), then col-packed K=1 outer-product matmuls broadcast it
    across partitions for the final numerator scaling.
  - All host-side input arrays are pre-tiled so every DMA descriptor is a
    contiguous >=1KB per-partition line.
"""

import numpy as np
import ml_dtypes
import bass_rust
import concourse.bass as bass
import concourse.mybir as mybir
import concourse.tile as tile
from concourse.bass_utils import run_bass_kernel_spmd
from concourse.masks import make_identity

AF = mybir.ActivationFunctionType
ALU = mybir.AluOpType
f32 = mybir.dt.float32
f16 = mybir.dt.float16
fp8 = mybir.dt.float8e4

B, T, S, D, H, DH = 4, 1024, 1024, 1024, 16, 64
P = 128
HL = 8            # heads per core
FL = HL * DH      # 512 local features
NEG = -240.0      # mask additive, exactly representable in fp8_e4m3
EPS = 1e-5
NCORES = 8
REPLICA_GROUPS = [[0, 1], [2, 3], [4, 5], [6, 7]]
NWARM = 14        # startup HAM-warming dummy matmuls
NBRIDGE = 20      # AR1-window dummy matmuls


# ---------------------------------------------------------------------------
# Workaround: this container's walrus build allows only ONE sync-wait per
# instruction ("Too many sync wait commands" in setupSyncWait).  After Tile
# scheduling, split any instruction's extra waits onto preceding NOPs on the
# same engine (the engine sequencer blocks on each wait in program order, so
# semantics are identical).
# ---------------------------------------------------------------------------
def _split_multi_waits(nc):
    cnt = 0
    for fn in nc.m.functions:
        for bb in fn.blocks:
            new = []
            changed = False
            for inst in bb.instructions:
                si = inst.sync_info
                if si is not None and len(si.on_wait) > 1:
                    changed = True
                    waits = list(si.on_wait)
                    for w in waits[:-1]:
                        cnt += 1
                        nop = mybir.InstNoOp(
                            name=f"I-waitsplit-{cnt}", ins=[], outs=[]
                        )
                        nop.engine = inst.engine
                        nop.sync_info = bass_rust.SyncInfo(
                            on_wait=[w], on_update=[]
                        )
                        new.append(nop)
                    inst.sync_info = bass_rust.SyncInfo(
                        on_wait=[waits[-1]], on_update=list(si.on_update)
                    )
                new.append(inst)
            if changed:
                bb.instructions = new
    return cnt


# ---------------------------------------------------------------------------
# Device program (identical on all 8 cores)
# ---------------------------------------------------------------------------
def _emit(nc, tc, qtl, ktl, vtl, wql, wkl, wvl, wol, biasl, out):
    from contextlib import ExitStack

    with ExitStack() as ctx:
        perst = ctx.enter_context(tc.tile_pool(name="perst", bufs=1))
        scratch = perst.tile([P, 512], f16)
        nc.vector.memset(scratch[:], 0.0)
        ones_bc = perst.tile([P, 64], f16)
        nc.gpsimd.memset(ones_bc[:], 1.0)
        ones_col = perst.tile([P, 8], f32)
        nc.gpsimd.memset(ones_col[:], 1.0)
        eps_t = perst.tile([P, 1], f32)
        nc.gpsimd.memset(eps_t[:], EPS)
        ident8 = perst.tile([P, P], fp8)
        make_identity(nc, ident8[:])
        ident16 = perst.tile([P, P], f16)
        make_identity(nc, ident16[:])
        zball = perst.tile([P, 64], f32)   # exp accum per (i, f, th)
        nc.gpsimd.memset(zball[:], 0.0)
        zloc = perst.tile([P, 8], f32)
        zsink = perst.tile([P, 8], f32)
        zg = perst.tile([P, 8], f32)
        zinv = perst.tile([P, 8], f32)

        vaug_pool = ctx.enter_context(tc.tile_pool(name="vaugp", bufs=1))
        vaug = [vaug_pool.tile([P, HL, 65], f16, name=f"vaug{i}")
                for i in range(8)]
        drp = ctx.enter_context(tc.tile_pool(name="drp", bufs=1, space="DRAM"))
        # Z-allreduce split 5/3: AV over s-tiles 0-4 only needs the first
        # AR (triggered after i=4), leaving just s5-7 exposed to the second
        # AR's ~30us latency.
        AR_LO, AR_HI = 5, 3
        din = [drp.tile([P, AR_LO], f32, name="din0"),
               drp.tile([P, AR_HI], f32, name="din1")]
        dout = [drp.tile([P, AR_LO], f32, name="dout0"),
                drp.tile([P, AR_HI], f32, name="dout1")]
        ddin = drp.tile([P, 4], f32, name="ddin")
        ddout = drp.tile([P, 4], f32, name="ddout")
        # dummy AllReduce: the first CC trigger pays ~11.5us of one-time
        # stream-init cost; swallow it during phase A so the real ARs are
        # prompt.
        nc.gpsimd.dma_start(ddin[:], ones_col[:, 0:4])
        nc.gpsimd.collective_compute(
            "AllReduce", ALU.add, replica_groups=REPLICA_GROUPS,
            ins=[ddin.opt()], outs=[ddout.opt()],
        )

        def _ar_range(sh):
            return (0, AR_LO) if sh == 0 else (AR_LO, 8)

        def _ar_trigger(sh):
            lo, hi = _ar_range(sh)
            nc.gpsimd.dma_start(din[sh][:], zloc[:, lo:hi])
            nc.gpsimd.collective_compute(
                "AllReduce", ALU.add, replica_groups=REPLICA_GROUPS,
                ins=[din[sh].opt()], outs=[dout[sh].opt()],
            )

        def _ar_consume(sh):
            lo, hi = _ar_range(sh)
            nc.gpsimd.dma_start(zg[:, lo:hi], dout[sh][:])
            nc.vector.reciprocal(zinv[:, lo:hi], zg[:, lo:hi])
            for i in range(lo, hi):
                nc.vector.tensor_scalar_mul(
                    vaug[i][:, :, 0:64], vaug[i][:, :, 0:64],
                    zinv[:, i:i + 1],
                )
                nc.vector.tensor_scalar_mul(
                    vaug[i][:, :, 64:65], ones_col[:, :, None],
                    zinv[:, i:i + 1],
                )

        # qh/kh live from phase A through the end of B1 scores; the pool
        # stays open (stack allocator) but is small.
        qk_pool = ctx.enter_context(tc.tile_pool(name="qk", bufs=1))
        qh = qk_pool.tile([P, 4, T], f16)      # qhT: [feat_loc, f_tile, t]
        kh = qk_pool.tile([P, 4, S], f16)      # khT: [feat_loc, f_tile, s]

        # V tiles + Wv sit below the phase-A pools in the stack so their
        # DMAs carry no write-after-read dependency on the freed A space.
        vpool = ctx.enter_context(tc.tile_pool(name="vpool", bufs=1))
        wvpool = ctx.enter_context(tc.tile_pool(name="wvpool", bufs=1))
        wv = wvpool.tile([P, 8, FL], f16)
        vt0 = vpool.tile([P, 8, 512], f16, tag="vt", name="vt0")

        # ---- phase A: q/k projections, with HAM warmup while DMAs land ----
        with ExitStack() as actx:
            apool = actx.enter_context(tc.tile_pool(name="apool", bufs=1))
            atp = actx.enter_context(tc.tile_pool(name="atp", bufs=1))
            apsum = actx.enter_context(
                tc.tile_pool(name="apsum", bufs=2, space="PSUM")
            )
            wps = apsum.tile([P, 512], f32, tag="warm")
            for _ in range(NWARM):
                nc.tensor.matmul(
                    wps[:], scratch[:, 0:128], scratch[:],
                    start=True, stop=True,
                )
            qt = atp.tile([P, 8, T], f16)
            kt = atp.tile([P, 8, S], f16)
            wq = [apool.tile([P, 8, P], f16, name=f"wq{f}") for f in range(4)]
            wk = [apool.tile([P, 8, P], f16, name=f"wk{f}") for f in range(4)]
            # first matmul needs wq0 + qt[:, 0:2] only; stream the rest behind
            nc.sync.dma_start(wq[0][:], wql[:, 0])
            nc.sync.dma_start(qt[:, 0:2, :], qtl[:, 0:2, :])
            nc.sync.dma_start(qt[:, 2:4, :], qtl[:, 2:4, :])
            nc.sync.dma_start(wq[1][:], wql[:, 1])
            nc.sync.dma_start(qt[:, 4:6, :], qtl[:, 4:6, :])
            nc.sync.dma_start(qt[:, 6:8, :], qtl[:, 6:8, :])
            nc.sync.dma_start(wq[2][:], wql[:, 2])
            nc.sync.dma_start(wq[3][:], wql[:, 3])

            for f in range(4):
                # kt + wk stream in behind the Q-projection matmuls
                nc.sync.dma_start(kt[:, 2 * f:2 * f + 2, :],
                                  ktl[:, 2 * f:2 * f + 2, :])
                nc.sync.dma_start(wk[f][:], wkl[:, f])
                ps = apsum.tile([P, 2, 512], f32, tag="qk")
                for c in range(8):
                    lw = wq[f][:, c, :]
                    nc.tensor.matmul(
                        ps[:, 0, :], lw, qt[:, c, 0:512],
                        start=(c == 0), stop=(c == 7),
                    )
                    nc.tensor.matmul(
                        ps[:, 1, :], lw, qt[:, c, 512:1024],
                        start=(c == 0), stop=(c == 7),
                    )
                # DH**-0.5 folded into wql host-side; evac on ACT (idle here)
                nc.scalar.copy(
                    qh[:, f, :].rearrange("p (b t) -> p b t", b=2), ps[:]
                )
            for f in range(4):
                ps = apsum.tile([P, 2, 512], f32, tag="qk")
                for c in range(8):
                    lw = wk[f][:, c, :]
                    nc.tensor.matmul(
                        ps[:, 0, :], lw, kt[:, c, 0:512],
                        start=(c == 0), stop=(c == 7),
                    )
                    nc.tensor.matmul(
                        ps[:, 1, :], lw, kt[:, c, 512:1024],
                        start=(c == 0), stop=(c == 7),
                    )
                nc.scalar.copy(
                    kh[:, f, :].rearrange("p (b s) -> p b s", b=2), ps[:]
                )

        # exp-weight buffer: opened after A so its 128KB/partition reuses
        # the qt/kt space; D1 accumulators + Wo likewise live here.
        wbuf_pool = ctx.enter_context(tc.tile_pool(name="wbufp", bufs=1))
        wre = wbuf_pool.tile([P, 8, HL, T], f16)
        d1pool = ctx.enter_context(tc.tile_pool(name="d1pool", bufs=1))
        tmpA = [d1pool.tile([P, 2, 512], f16, name=f"tmpA{h}")
                for h in range(HL)]
        # B-phase streaming pools (closed right after B to free their space
        # for the D/E pools -- opened last so the stack pops cleanly)
        bstuff = ctx.enter_context(ExitStack())
        ebp = bstuff.enter_context(tc.tile_pool(name="ebp", bufs=3))
        ebt = {}
        for i in range(8):
            for f in range(4):
                ebt[(i, f)] = ebp.tile(
                    [P, 4, 512], fp8, tag="eb", name=f"eb{i}_{f}"
                )
        stgp = bstuff.enter_context(tc.tile_pool(name="stgp", bufs=2))
        # B0-front prefetches (the DMA queue drains these behind phase A's
        # transfers, overlapping the A matmuls)
        nc.sync.dma_start(ebt[(0, 0)][:], biasl[0, 0])
        nc.sync.dma_start(ebt[(0, 1)][:], biasl[0, 1])
        nc.gpsimd.dma_start(vt0[:, 0:4, :], vtl[:, 0:4, 0:512])
        nc.gpsimd.dma_start(vt0[:, 4:8, :], vtl[:, 4:8, 0:512])
        nc.gpsimd.dma_start(wv[:, 0:4, :], wvl[:, 0:4, :])
        nc.gpsimd.dma_start(wv[:, 4:8, :], wvl[:, 4:8, :])

        # ---- phases B+C chunked by s-half, V-projection matmuls emitted as
        # PE filler inside the B0 loop, with a split Z-allreduce. ----
        # Bias-add alternates between two paths per (i, f) group so neither
        # engine paces the phase: even f -> PE identity-matmul adds (exp
        # straight from PSUM); odd f -> DVE tensor_add into an f16 staging
        # tile (exp from SBUF, N=2048).
        def _score_mms(ps, i, f, th, stop):
            for u in range(2):
                po = u * 64
                nc.tensor.matmul(
                    ps[:, u, :],
                    kh[po:po + 64, f, i * P:(i + 1) * P],
                    qh[po:po + 64, f, th * 512:(th + 1) * 512],
                    start=True, stop=stop,
                )

        def _unit_pe(bpsum, i, f):
            zi = (i * 4 + f) * 2
            for th in range(2):
                ps = bpsum.tile([P, 2, 512], f32, tag="eps")
                _score_mms(ps, i, f, th, stop=False)
                for u in range(2):
                    nc.tensor.matmul(
                        ps[:, u, :], ident8[:], bt_cur[0][:, th * 2 + u, :],
                        start=False, stop=True,
                    )
                nc.scalar.activation(
                    wre[:, i, 2 * f:2 * f + 2, th * 512:(th + 1) * 512],
                    ps[:], AF.Exp, accum_out=zball[:, zi + th:zi + th + 1],
                )

        def _unit_dve(bpsum, i, f):
            zi = (i * 4 + f) * 2
            stg_t = stgp.tile([P, 2, 2, 512], f16, tag="stg")
            for th in range(2):
                ps = bpsum.tile([P, 2, 512], f32, tag="eps")
                _score_mms(ps, i, f, th, stop=True)
                nc.vector.tensor_add(
                    stg_t[:, :, th, :], ps[:], bt_cur[0][:, th * 2:th * 2 + 2, :]
                )
            nc.scalar.activation(
                wre[:, i, 2 * f:2 * f + 2, :],
                stg_t[:].rearrange("p u a b -> p u (a b)"),
                AF.Exp, accum_out=zball[:, zi:zi + 1],
            )

        bt_cur = [None]

        def _c_chunk(cp, il, vt, crange):
            for c in crange:
                nc.tensor.matmul(
                    cp[:], vt[:, c, il * P:(il + 1) * P], wv[:, c, :],
                    start=(c == 0), stop=(c == 7),
                )

        def _c_evac(cp, i):
            nc.vector.tensor_copy(
                vaug[i][:, :, 0:64],
                cp[:].rearrange("p (h c) -> p h c", c=64),
            )

        with ExitStack() as bctx:
            bpsum = bctx.enter_context(
                tc.tile_pool(name="bpsum", bufs=3, space="PSUM")
            )
            cpsum = bctx.enter_context(
                tc.tile_pool(name="cpsum", bufs=1, space="PSUM")
            )
            dpsum = bctx.enter_context(
                tc.tile_pool(name="dpsum", bufs=1, space="PSUM")
            )
            # ---- B0 (+ interleaved C0) ----
            for i in range(4):
                cp = cpsum.tile([P, 512], f32, tag="vps", name=f"vps{i}")
                for f in range(4):
                    bt_cur[0] = ebt[(i, f)]
                    if not (i == 0 and f <= 1):
                        nc.sync.dma_start(bt_cur[0][:], biasl[i, f])
                    if f % 2 == 0:
                        _unit_pe(bpsum, i, f)
                    else:
                        _unit_dve(bpsum, i, f)
                    _c_chunk(cp, i, vt0, (2 * f, 2 * f + 1))
                    if f == 3:
                        _c_evac(cp, i)
                nc.scalar.activation(
                    zsink[:], zball[:, i * 8:(i + 1) * 8], AF.Copy,
                    accum_out=zloc[:, i:i + 1],
                )

            # ---- B1: no C filler (held back for the AR1 window) ----
            vt1 = vpool.tile([P, 8, 512], f16, tag="vt", name="vt1")
            nc.gpsimd.dma_start(vt1[:, 0:4, :], vtl[:, 0:4, 512:1024])
            nc.gpsimd.dma_start(vt1[:, 4:8, :], vtl[:, 4:8, 512:1024])
            for i in range(4, 8):
                if i == AR_LO:
                    _ar_trigger(0)
                for f in range(4):
                    bt_cur[0] = ebt[(i, f)]
                    nc.sync.dma_start(bt_cur[0][:], biasl[i, f])
                    if f % 2 == 0:
                        _unit_pe(bpsum, i, f)
                    else:
                        _unit_dve(bpsum, i, f)
                nc.scalar.activation(
                    zsink[:], zball[:, i * 8:(i + 1) * 8], AF.Copy,
                    accum_out=zloc[:, i:i + 1],
                )
            _ar_trigger(1)

            # held-back s-half-1 v-projection covers the AR1 window; it
            # must precede consume0, which scales vaug[4] (AR_LO=5).
            for il in range(4):
                i = 4 + il
                cp = cpsum.tile([P, 512], f32, tag="vps", name=f"vps{i}")
                _c_chunk(cp, il, vt1, range(8))
                _c_evac(cp, i)

            _ar_consume(0)

            # D1: s-tiles 0-3 AV chains for ALL heads (independent of the
            # second Z allreduce) -- PE filler behind B1 in the queue.  The
            # two t-halves chain into single-bank tiles from two pools so
            # consecutive heads pipeline without waiting on the ACT evac.
            for h in range(HL):
                psA0 = dpsum.tile([P, 512], f32, tag="avp")
                psA1 = cpsum.tile([P, 512], f32, tag="vps")
                for i in range(AR_LO):
                    lw = vaug[i][:, h, :]
                    nc.tensor.matmul(
                        psA0[0:65, :], lw, wre[:, i, h, 0:512],
                        start=(i == 0), stop=(i == AR_LO - 1),
                    )
                    nc.tensor.matmul(
                        psA1[0:65, :], lw, wre[:, i, h, 512:1024],
                        start=(i == 0), stop=(i == AR_LO - 1),
                    )
                nc.scalar.copy(tmpA[h][0:65, 0, :], psA0[0:65, :])
                nc.scalar.copy(tmpA[h][0:65, 1, :], psA1[0:65, :])

            _ar_consume(1)

            # dummy-matmul bridge keeps HAM warm across the AR1 wait
            for _ in range(NBRIDGE):
                br = dpsum.tile([P, 512], f32, tag="avp")
                nc.tensor.matmul(
                    br[:], scratch[:, 0:128], scratch[:],
                    start=True, stop=True,
                )

        bstuff.close()   # ebt/stg/vt/wv space freed for the D/E pools

        # ---- phase D2: AV s-tiles 4-7 + grouped renorm ----
        dstack = ctx.enter_context(ExitStack())
        dpool = dstack.enter_context(tc.tile_pool(name="dpool", bufs=1))
        wo = dpool.tile([P, 4, D], f16)
        for ch in range(4):
            nc.sync.dma_start(wo[:, ch, :], wol[:, ch])
        # r-rows for all 8 heads: head h -> partition 32*(h%4), block h//4.
        # memset to 1 so Ln on unused rows is finite.  f16 + in-place
        # Ln/Exp; the renormed numerators overwrite tmpA in place, with
        # tmpA[2j] doubling as aop[j] for the E matmuls.
        rcol = dpool.tile([P, 2, T], f16)
        nc.gpsimd.memset(rcol[:], 1.0)
        aop = [tmpA[2 * j][:].rearrange("p a b -> p (a b)") for j in range(4)]

        d2stack = ctx.enter_context(ExitStack())
        d2psum = d2stack.enter_context(
            tc.tile_pool(name="d2psum", bufs=2, space="PSUM")
        )
        pbp = d2stack.enter_context(
            tc.tile_pool(name="pbp", bufs=1, space="PSUM")
        )
        for h in range(HL):
            psB = d2psum.tile([P, 2, 512], f32, tag="avp2")
            for i in range(5, 8):
                lw = vaug[i][:, h, :]
                nc.tensor.matmul(
                    psB[0:65, 0, :], lw, wre[:, i, h, 0:512],
                    start=(i == 5), stop=False,
                )
                nc.tensor.matmul(
                    psB[0:65, 1, :], lw, wre[:, i, h, 512:1024],
                    start=(i == 5), stop=False,
                )
            # merge the D1 partial via an identity matmul (PE adds are
            # cheap; DVE is the scarcer engine here), re-evacuate the summed
            # numerator on ACT, then scatter the r-row into rcol's
            # 32-aligned layout
            for th in range(2):
                nc.tensor.matmul(
                    psB[0:65, th, :], ident16[0:65, 0:65],
                    tmpA[h][0:65, th, :], start=False, stop=True,
                )
            nc.scalar.copy(tmpA[h][0:65, :, :], psB[0:65, :, :])
            rp = 32 * (h % 4)
            nc.vector.tensor_copy(
                rcol[rp:rp + 1, h // 4, :],
                tmpA[h][64:65, :, :].rearrange("p a b -> p (a b)"),
            )
            if h % 4 == 3:
                g = h // 4
                # 1/(r+eps) for this 4-head group in two ACT calls
                nc.scalar.activation(
                    rcol[:, g, :], rcol[:, g, :], AF.Ln, bias=eps_t[:]
                )
                nc.scalar.activation(
                    rcol[:, g, :], rcol[:, g, :], AF.Exp, scale=-1.0
                )
                for j in (2 * g, 2 * g + 1):
                    pb = pbp.tile([P, 2, 512], f32, tag="bcp")
                    for u in range(2):
                        hh = 2 * j + u
                        rp2 = 32 * (hh % 4)
                        for th in range(2):
                            nc.tensor.matmul(
                                pb[u * 64:(u + 1) * 64, th, :],
                                ones_bc[rp2:rp2 + 1, :],
                                rcol[rp2:rp2 + 1, g,
                                     th * 512:(th + 1) * 512],
                                start=True, stop=True,
                                tile_position=(rp2, u * 64),
                            )
                    # u=1 runs first: it reads tmpA[2j+1] and writes rows
                    # 64:128 of tmpA[2j] (aop[j]), so the u=0 in-place
                    # multiply on rows 0:64 never conflicts.
                    for u in (1, 0):
                        hh = 2 * j + u
                        nc.vector.tensor_mul(
                            out=aop[j][u * 64:(u + 1) * 64, :],
                            in0=tmpA[hh][0:64, :, :].rearrange(
                                "p a b -> p (a b)"),
                            in1=pb[u * 64:(u + 1) * 64, :, :].rearrange(
                                "p a b -> p (a b)"),
                        )
        d2stack.close()

        # ---- phase E: output projection (row-parallel partial; the pair
        # sum happens in the host-side unshard). ----
        estack = ctx.enter_context(ExitStack())
        epool = estack.enter_context(tc.tile_pool(name="epool", bufs=2))
        opsum = estack.enter_context(
            tc.tile_pool(name="opsum", bufs=2, space="PSUM")
        )
        for tch in range(8):
            pso = opsum.tile([P, 2, 512], f32, tag="outp")
            for j in range(4):
                lw = aop[j][:, tch * P:(tch + 1) * P]
                nc.tensor.matmul(
                    pso[:, 0, :], lw, wo[:, j, 0:512],
                    start=(j == 0), stop=(j == 3),
                )
                nc.tensor.matmul(
                    pso[:, 1, :], lw, wo[:, j, 512:1024],
                    start=(j == 0), stop=(j == 3),
                )
            ot = epool.tile([P, 2, 512], f16, tag="ot")
            nc.scalar.copy(ot[:], pso[:])
            nc.sync.dma_start(
                out[tch * P:(tch + 1) * P, :],
                ot[:].rearrange("p a b -> p (a b)"),
            )


def build_nc():
    nc = bass.Bass(num_devices=NCORES)
    qtl = nc.dram_tensor("qtl", [P, 8, T], f16, kind="ExternalInput")
    ktl = nc.dram_tensor("ktl", [P, 8, S], f16, kind="ExternalInput")
    vtl = nc.dram_tensor("vtl", [P, 8, S], f16, kind="ExternalInput")
    wql = nc.dram_tensor("wql", [P, 4, 8, P], f16, kind="ExternalInput")
    wkl = nc.dram_tensor("wkl", [P, 4, 8, P], f16, kind="ExternalInput")
    wvl = nc.dram_tensor("wvl", [P, 8, FL], f16, kind="ExternalInput")
    wol = nc.dram_tensor("wol", [P, 4, D], f16, kind="ExternalInput")
    biasl = nc.dram_tensor("biasl", [8, 4, P, 4, 512], fp8,
                           kind="ExternalInput")
    out = nc.dram_tensor("out", [T, D], f16, kind="ExternalOutput")
    with tile.TileContext(nc) as tc:
        _emit(nc, tc, qtl, ktl, vtl, wql, wkl, wvl, wol, biasl, out)
    _split_multi_waits(nc)
    return nc


_NC_CACHE = {}


def _get_nc():
    if "nc" not in _NC_CACHE:
        _NC_CACHE["nc"] = build_nc()
    return _NC_CACHE["nc"]


# ---------------------------------------------------------------------------
# Host wrapper
# ---------------------------------------------------------------------------
def _prep_inputs(q, k, v, attn_mask, attn_bias, Wq, Wk, Wv, Wo):
    q = np.asarray(q, np.float32)
    k = np.asarray(k, np.float32)
    v = np.asarray(v, np.float32)
    mask = np.asarray(attn_mask)
    bias = np.asarray(attn_bias, np.float32)
    WqT = np.asarray(Wq, np.float32).T * np.float32(DH ** -0.5)
    WkT = np.ascontiguousarray(np.asarray(Wk, np.float32).T)
    WvT = np.ascontiguousarray(np.asarray(Wv, np.float32).T)
    WoT = np.ascontiguousarray(np.asarray(Wo, np.float32).T)

    def _tile_qkv(x):     # [T, D] -> [p, c, t] f16
        return np.ascontiguousarray(
            x.T.reshape(8, P, T).transpose(1, 0, 2)).astype(np.float16)

    def _tile_wqk(wt, hh):   # [D, D] col-slice -> [p, f, o, g] f16
        sl = wt[:, hh * FL:(hh + 1) * FL]
        return np.ascontiguousarray(
            sl.reshape(8, P, 4, P).transpose(1, 2, 0, 3)).astype(np.float16)

    # bias + additive mask (exp underflows to 0 on device), per head-half,
    # tiled [i, f, p, th*2+u, t'] with h = 2f+u, s = i*128+p, t = th*512+t'
    pre = bias + np.where(mask, np.float32(NEG), np.float32(0.0))[None]
    w_hh = []
    for hh in range(2):
        ph = pre[hh * HL:(hh + 1) * HL]      # [8h, t, s]
        arr = ph.reshape(4, 2, 2, 512, 8, P).transpose(4, 0, 5, 2, 1, 3)
        arr = np.clip(arr, -240.0, 240.0)
        bias8 = np.ascontiguousarray(arr).reshape(8, 4, P, 4, 512) \
            .astype(ml_dtypes.float8_e4m3)
        w_hh.append({
            "wql": _tile_wqk(WqT, hh),
            "wkl": _tile_wqk(WkT, hh),
            "wvl": np.ascontiguousarray(
                WvT[:, hh * FL:(hh + 1) * FL].reshape(8, P, FL)
                .transpose(1, 0, 2)).astype(np.float16),
            "wol": np.ascontiguousarray(
                WoT[hh * FL:(hh + 1) * FL, :].reshape(4, P, D)
                .transpose(1, 0, 2)).astype(np.float16),
            "biasl": bias8,
        })

    in_maps = []
    for c in range(NCORES):
        b, hh = c // 2, c % 2
        if hh == 0:
            qt, kt, vt = _tile_qkv(q[b]), _tile_qkv(k[b]), _tile_qkv(v[b])
        in_maps.append({
            "qtl": qt, "ktl": kt, "vtl": vt, **w_hh[hh],
        })
    return in_maps


def run(inputs, trace=False):
    nc = _get_nc()
    in_maps = _prep_inputs(**inputs)
    res = run_bass_kernel_spmd(
        nc, in_maps, core_ids=list(range(NCORES)), trace=trace,
    )
    full = np.empty((B, T, D), np.float32)
    for b in range(B):
        full[b] = (
            np.asarray(res.results[2 * b]["out"], dtype=np.float32)
            + np.asarray(res.results[2 * b + 1]["out"], dtype=np.float32)
        )
    return full, res


def kernel(**inputs) -> np.ndarray:
    full, _ = run(inputs, trace=False)
    return full


# revision 10
# speedup vs baseline: 1.0864x; 1.0179x over previous
"""Trainium2 Bass kernel for nn_MultiHeadAttention_84318797955257.

Inverted-softmax multi-head attention, 8-core SPMD.

  - Sharding: 4 batches x 2 head-halves (each core: 1 batch, all 1024 query
    rows, 8 of 16 heads, full S).  Wq/Wk/Wv are column-sliced and Wo
    row-sliced per head-half, so no projection work is duplicated anywhere
    (the previous batch x T-half layout duplicated the K/V projections on
    both cores of a pair).  The row-parallel Wo partial sums are added
    during the host-side unshard (one f16 partial per core, pair-summed in
    f32); the only device collective is the tiny inverted-softmax
    denominator Z[b,s]: a [128,4] f32 AllReduce per s-half over core pairs.
  - Mask folded into bias host-side as -240 additive fp8_e4m3 (exp
    underflows to 0, matching the reference's where(mask,0)); DH**-0.5
    folded into Wq.  Bias streams as fp8 (8.4MB/core, half the f16 cost).
  - Scores are computed in [s, t] layout (eT = khT.T @ qhT per head) so the
    AV matmul needs no transpose.  Per-head K=64 score matmuls are packed
    pairwise into disjoint PE row-groups and run concurrently.  The bias
    lands via DVE tensor_add (PSUM f32 + fp8 -> f16 staging); this frees
    the PE from the 128 identity-matmul bias adds the old kernel used
    (~28us of PE), and exp then runs on ACT from SBUF in N=2048 chunks
    (2 heads x 1024 t) with accum_out producing the Z partial sums.
  - ~20 dummy matmuls on a zero scratch tile at kernel start keep the PE
    busy while the first DMAs land, flipping the HAM clock-gate to 8/8
    before real work starts; a short dummy-matmul bridge between D1 and D2
    keeps it warm across the second AllReduce wait.
  - 1/Z is folded into V rows (v' = v/Z); an extra ones-column in V yields
    the S-renorm denominator r as row 64 of the AV output.  AV runs in two
    waves: D1 (s-tiles 0-3, after AR0) is emitted behind the B1 score loop
    as PE filler; D2 (s-tiles 4-7, after AR1) merges the D1 partial via a
    DVE add.  r-rows are collected cross-partition into a
    [{0,32,64,96} x 2] layout so one Ln + one Exp per 4-head group computes
    1/(r+eps), then col-packed K=1 outer-product matmuls broadcast it
    across partitions for the final numerator scaling.
  - All host-side input arrays are pre-tiled so every DMA descriptor is a
    contiguous >=1KB per-partition line.
"""

import numpy as np
import ml_dtypes
import bass_rust
import concourse.bass as bass
import concourse.mybir as mybir
import concourse.tile as tile
from concourse.bass_utils import run_bass_kernel_spmd
from concourse.masks import make_identity

AF = mybir.ActivationFunctionType
ALU = mybir.AluOpType
f32 = mybir.dt.float32
f16 = mybir.dt.float16
fp8 = mybir.dt.float8e4

B, T, S, D, H, DH = 4, 1024, 1024, 1024, 16, 64
P = 128
HL = 8            # heads per core
FL = HL * DH      # 512 local features
NEG = -240.0      # mask additive, exactly representable in fp8_e4m3
EPS = 1e-5
NCORES = 8
REPLICA_GROUPS = [[0, 1], [2, 3], [4, 5], [6, 7]]
NWARM = 14        # startup HAM-warming dummy matmuls
NBRIDGE = 20      # AR1-window dummy matmuls


# ---------------------------------------------------------------------------
# Workaround: this container's walrus build allows only ONE sync-wait per
# instruction ("Too many sync wait commands" in setupSyncWait).  After Tile
# scheduling, split any instruction's extra waits onto preceding NOPs on the
# same engine (the engine sequencer blocks on each wait in program order, so
# semantics are identical).
# ---------------------------------------------------------------------------
def _split_multi_waits(nc):
    cnt = 0
    for fn in nc.m.functions:
        for bb in fn.blocks:
            new = []
            changed = False
            for inst in bb.instructions:
                si = inst.sync_info
                if si is not None and len(si.on_wait) > 1:
                    changed = True
                    waits = list(si.on_wait)
                    for w in waits[:-1]:
                        cnt += 1
                        nop = mybir.InstNoOp(
                            name=f"I-waitsplit-{cnt}", ins=[], outs=[]
                        )
                        nop.engine = inst.engine
                        nop.sync_info = bass_rust.SyncInfo(
                            on_wait=[w], on_update=[]
                        )
                        new.append(nop)
                    inst.sync_info = bass_rust.SyncInfo(
                        on_wait=[waits[-1]], on_update=list(si.on_update)
                    )
                new.append(inst)
            if changed:
                bb.instructions = new
    return cnt


# ---------------------------------------------------------------------------
# Device program (identical on all 8 cores)
# ---------------------------------------------------------------------------
def _emit(nc, tc, qtl, ktl, vtl, wql, wkl, wvl, wol, biasl, out):
    from contextlib import ExitStack

    with ExitStack() as ctx:
        perst = ctx.enter_context(tc.tile_pool(name="perst", bufs=1))
        scratch = perst.tile([P, 512], f16)
        nc.vector.memset(scratch[:], 0.0)
        ones_bc = perst.tile([P, 64], f16)
        nc.gpsimd.memset(ones_bc[:], 1.0)
        ones_col = perst.tile([P, 8], f32)
        nc.gpsimd.memset(ones_col[:], 1.0)
        eps_t = perst.tile([P, 1], f32)
        nc.gpsimd.memset(eps_t[:], EPS)
        ident8 = perst.tile([P, P], fp8)
        make_identity(nc, ident8[:])
        ident16 = perst.tile([P, P], f16)
        make_identity(nc, ident16[:])
        zball = perst.tile([P, 64], f32)   # exp accum per (i, f, th)
        nc.gpsimd.memset(zball[:], 0.0)
        zloc = perst.tile([P, 8], f32)
        zsink = perst.tile([P, 8], f32)
        zg = perst.tile([P, 8], f32)
        zinv = perst.tile([P, 8], f32)

        vaug_pool = ctx.enter_context(tc.tile_pool(name="vaugp", bufs=1))
        vaug = [vaug_pool.tile([P, HL, 65], f16, name=f"vaug{i}")
                for i in range(8)]
        drp = ctx.enter_context(tc.tile_pool(name="drp", bufs=1, space="DRAM"))
        # Z-allreduce split 5/3: AV over s-tiles 0-4 only needs the first
        # AR (triggered after i=4), leaving just s5-7 exposed to the second
        # AR's ~30us latency.
        AR_LO, AR_HI = 5, 3
        din = [drp.tile([P, AR_LO], f32, name="din0"),
               drp.tile([P, AR_HI], f32, name="din1")]
        dout = [drp.tile([P, AR_LO], f32, name="dout0"),
                drp.tile([P, AR_HI], f32, name="dout1")]
        ddin = drp.tile([P, 4], f32, name="ddin")
        ddout = drp.tile([P, 4], f32, name="ddout")
        # dummy AllReduce: the first CC trigger pays ~11.5us of one-time
        # stream-init cost; swallow it during phase A so the real ARs are
        # prompt.
        nc.gpsimd.dma_start(ddin[:], ones_col[:, 0:4])
        nc.gpsimd.collective_compute(
            "AllReduce", ALU.add, replica_groups=REPLICA_GROUPS,
            ins=[ddin.opt()], outs=[ddout.opt()],
        )

        def _ar_range(sh):
            return (0, AR_LO) if sh == 0 else (AR_LO, 8)

        def _ar_trigger(sh):
            lo, hi = _ar_range(sh)
            nc.gpsimd.dma_start(din[sh][:], zloc[:, lo:hi])
            nc.gpsimd.collective_compute(
                "AllReduce", ALU.add, replica_groups=REPLICA_GROUPS,
                ins=[din[sh].opt()], outs=[dout[sh].opt()],
            )

        def _ar_consume(sh):
            lo, hi = _ar_range(sh)
            nc.gpsimd.dma_start(zg[:, lo:hi], dout[sh][:])
            nc.vector.reciprocal(zinv[:, lo:hi], zg[:, lo:hi])
            for i in range(lo, hi):
                nc.vector.tensor_scalar_mul(
                    vaug[i][:, :, 0:64], vaug[i][:, :, 0:64],
                    zinv[:, i:i + 1],
                )
                nc.vector.tensor_scalar_mul(
                    vaug[i][:, :, 64:65], ones_col[:, :, None],
                    zinv[:, i:i + 1],
                )

        # qh/kh live from phase A through the end of B1 scores; the pool
        # stays open (stack allocator) but is small.
        qk_pool = ctx.enter_context(tc.tile_pool(name="qk", bufs=1))
        qh = qk_pool.tile([P, 4, T], f16)      # qhT: [feat_loc, f_tile, t]
        kh = qk_pool.tile([P, 4, S], f16)      # khT: [feat_loc, f_tile, s]

        # V tiles + Wv sit below the phase-A pools in the stack so their
        # DMAs carry no write-after-read dependency on the freed A space.
        vpool = ctx.enter_context(tc.tile_pool(name="vpool", bufs=1))
        wvpool = ctx.enter_context(tc.tile_pool(name="wvpool", bufs=1))
        wv = wvpool.tile([P, 8, FL], f16)
        vt0 = vpool.tile([P, 8, 512], f16, tag="vt", name="vt0")

        # ---- phase A: q/k projections, with HAM warmup while DMAs land ----
        with ExitStack() as actx:
            apool = actx.enter_context(tc.tile_pool(name="apool", bufs=1))
            atp = actx.enter_context(tc.tile_pool(name="atp", bufs=1))
            apsum = actx.enter_context(
                tc.tile_pool(name="apsum", bufs=2, space="PSUM")
            )
            wps = apsum.tile([P, 512], f32, tag="warm")
            for _ in range(NWARM):
                nc.tensor.matmul(
                    wps[:], scratch[:, 0:128], scratch[:],
                    start=True, stop=True,
                )
            qt = atp.tile([P, 8, T], f16)
            kt = atp.tile([P, 8, S], f16)
            wq = [apool.tile([P, 8, P], f16, name=f"wq{f}") for f in range(4)]
            wk = [apool.tile([P, 8, P], f16, name=f"wk{f}") for f in range(4)]
            # first matmul needs wq0 + qt[:, 0:2] only; stream the rest behind
            nc.sync.dma_start(wq[0][:], wql[:, 0])
            nc.sync.dma_start(qt[:, 0:2, :], qtl[:, 0:2, :])
            nc.sync.dma_start(qt[:, 2:4, :], qtl[:, 2:4, :])
            nc.sync.dma_start(wk[0][:], wkl[:, 0])
            nc.sync.dma_start(kt[:, 0:2, :], ktl[:, 0:2, :])
            nc.sync.dma_start(wq[1][:], wql[:, 1])
            nc.sync.dma_start(qt[:, 4:6, :], qtl[:, 4:6, :])
            nc.sync.dma_start(kt[:, 2:4, :], ktl[:, 2:4, :])
            nc.sync.dma_start(wq[2][:], wql[:, 2])
            nc.sync.dma_start(qt[:, 6:8, :], qtl[:, 6:8, :])
            nc.sync.dma_start(wq[3][:], wql[:, 3])

            for f in range(4):
                # remaining kt + wk stream in behind the Q-proj matmuls
                if f >= 2:
                    nc.sync.dma_start(kt[:, 2 * f:2 * f + 2, :],
                                      ktl[:, 2 * f:2 * f + 2, :])
                if f >= 1:
                    nc.sync.dma_start(wk[f][:], wkl[:, f])
                ps = apsum.tile([P, 2, 512], f32, tag="qk")
                for c in range(8):
                    lw = wq[f][:, c, :]
                    nc.tensor.matmul(
                        ps[:, 0, :], lw, qt[:, c, 0:512],
                        start=(c == 0), stop=(c == 7),
                    )
                    nc.tensor.matmul(
                        ps[:, 1, :], lw, qt[:, c, 512:1024],
                        start=(c == 0), stop=(c == 7),
                    )
                # DH**-0.5 folded into wql host-side; evac on ACT (idle here)
                nc.scalar.copy(
                    qh[:, f, :].rearrange("p (b t) -> p b t", b=2), ps[:]
                )
            for f in range(4):
                ps = apsum.tile([P, 2, 512], f32, tag="qk")
                for c in range(8):
                    lw = wk[f][:, c, :]
                    nc.tensor.matmul(
                        ps[:, 0, :], lw, kt[:, c, 0:512],
                        start=(c == 0), stop=(c == 7),
                    )
                    nc.tensor.matmul(
                        ps[:, 1, :], lw, kt[:, c, 512:1024],
                        start=(c == 0), stop=(c == 7),
                    )
                nc.scalar.copy(
                    kh[:, f, :].rearrange("p (b s) -> p b s", b=2), ps[:]
                )

        # exp-weight buffer: opened after A so its 128KB/partition reuses
        # the qt/kt space; D1 accumulators + Wo likewise live here.
        wbuf_pool = ctx.enter_context(tc.tile_pool(name="wbufp", bufs=1))
        wre = wbuf_pool.tile([P, 8, HL, T], f16)
        d1pool = ctx.enter_context(tc.tile_pool(name="d1pool", bufs=1))
        tmpA = [d1pool.tile([P, 2, 512], f16, name=f"tmpA{h}")
                for h in range(HL)]
        # B-phase streaming pools (closed right after B to free their space
        # for the D/E pools -- opened last so the stack pops cleanly)
        bstuff = ctx.enter_context(ExitStack())
        ebp = bstuff.enter_context(tc.tile_pool(name="ebp", bufs=3))
        ebt = {}
        for i in range(8):
            for f in range(4):
                ebt[(i, f)] = ebp.tile(
                    [P, 4, 512], fp8, tag="eb", name=f"eb{i}_{f}"
                )
        stgp = bstuff.enter_context(tc.tile_pool(name="stgp", bufs=2))
        # B0-front prefetches (the DMA queue drains these behind phase A's
        # transfers, overlapping the A matmuls)
        nc.sync.dma_start(ebt[(0, 0)][:], biasl[0, 0])
        nc.sync.dma_start(ebt[(0, 1)][:], biasl[0, 1])
        nc.gpsimd.dma_start(vt0[:, 0:4, :], vtl[:, 0:4, 0:512])
        nc.gpsimd.dma_start(vt0[:, 4:8, :], vtl[:, 4:8, 0:512])
        nc.gpsimd.dma_start(wv[:, 0:4, :], wvl[:, 0:4, :])
        nc.gpsimd.dma_start(wv[:, 4:8, :], wvl[:, 4:8, :])

        # ---- phases B+C chunked by s-half, V-projection matmuls emitted as
        # PE filler inside the B0 loop, with a split Z-allreduce. ----
        # Bias-add alternates between two paths per (i, f) group so neither
        # engine paces the phase: even f -> PE identity-matmul adds (exp
        # straight from PSUM); odd f -> DVE tensor_add into an f16 staging
        # tile (exp from SBUF, N=2048).
        def _score_mms(ps, i, f, th, stop):
            for u in range(2):
                po = u * 64
                nc.tensor.matmul(
                    ps[:, u, :],
                    kh[po:po + 64, f, i * P:(i + 1) * P],
                    qh[po:po + 64, f, th * 512:(th + 1) * 512],
                    start=True, stop=stop,
                )

        def _unit_pe(bpsum, i, f):
            zi = (i * 4 + f) * 2
            for th in range(2):
                ps = bpsum.tile([P, 2, 512], f32, tag="eps")
                _score_mms(ps, i, f, th, stop=False)
                for u in range(2):
                    nc.tensor.matmul(
                        ps[:, u, :], ident8[:], bt_cur[0][:, th * 2 + u, :],
                        start=False, stop=True,
                    )
                nc.scalar.activation(
                    wre[:, i, 2 * f:2 * f + 2, th * 512:(th + 1) * 512],
                    ps[:], AF.Exp, accum_out=zball[:, zi + th:zi + th + 1],
                )

        def _unit_dve(bpsum, i, f):
            zi = (i * 4 + f) * 2
            stg_t = stgp.tile([P, 2, 2, 512], f16, tag="stg")
            for th in range(2):
                ps = bpsum.tile([P, 2, 512], f32, tag="eps")
                _score_mms(ps, i, f, th, stop=True)
                nc.vector.tensor_add(
                    stg_t[:, :, th, :], ps[:], bt_cur[0][:, th * 2:th * 2 + 2, :]
                )
            nc.scalar.activation(
                wre[:, i, 2 * f:2 * f + 2, :],
                stg_t[:].rearrange("p u a b -> p u (a b)"),
                AF.Exp, accum_out=zball[:, zi:zi + 1],
            )

        bt_cur = [None]

        def _c_chunk(cp, il, vt, crange):
            for c in crange:
                nc.tensor.matmul(
                    cp[:], vt[:, c, il * P:(il + 1) * P], wv[:, c, :],
                    start=(c == 0), stop=(c == 7),
                )

        def _c_evac(cp, i):
            nc.vector.tensor_copy(
                vaug[i][:, :, 0:64],
                cp[:].rearrange("p (h c) -> p h c", c=64),
            )

        with ExitStack() as bctx:
            bpsum = bctx.enter_context(
                tc.tile_pool(name="bpsum", bufs=3, space="PSUM")
            )
            cpsum = bctx.enter_context(
                tc.tile_pool(name="cpsum", bufs=1, space="PSUM")
            )
            dpsum = bctx.enter_context(
                tc.tile_pool(name="dpsum", bufs=1, space="PSUM")
            )
            # ---- B0 (+ interleaved C0) ----
            for i in range(4):
                for f in range(4):
                    bt_cur[0] = ebt[(i, f)]
                    if not (i == 0 and f <= 1):
                        nc.sync.dma_start(bt_cur[0][:], biasl[i, f])
                    if f % 2 == 0:
                        _unit_pe(bpsum, i, f)
                    else:
                        _unit_dve(bpsum, i, f)
                nc.scalar.activation(
                    zsink[:], zball[:, i * 8:(i + 1) * 8], AF.Copy,
                    accum_out=zloc[:, i:i + 1],
                )

            # ---- B1: no C filler (held back for the AR1 window) ----
            vt1 = vpool.tile([P, 8, 512], f16, tag="vt", name="vt1")
            nc.gpsimd.dma_start(vt1[:, 0:4, :], vtl[:, 0:4, 512:1024])
            nc.gpsimd.dma_start(vt1[:, 4:8, :], vtl[:, 4:8, 512:1024])
            for i in range(4, 8):
                if i == AR_LO:
                    _ar_trigger(0)
                for f in range(4):
                    bt_cur[0] = ebt[(i, f)]
                    nc.sync.dma_start(bt_cur[0][:], biasl[i, f])
                    if f % 2 == 0:
                        _unit_pe(bpsum, i, f)
                    else:
                        _unit_dve(bpsum, i, f)
                nc.scalar.activation(
                    zsink[:], zball[:, i * 8:(i + 1) * 8], AF.Copy,
                    accum_out=zloc[:, i:i + 1],
                )
            _ar_trigger(1)

            # ALL V-projection chains are emitted here: the scheduler
            # hoists them into B1's ACT-paced idle slots, and the leftovers
            # cover the first AllReduce's latency.  They must precede
            # consume0, which scales vaug[0..4].
            for i in range(8):
                cp = cpsum.tile([P, 512], f32, tag="vps", name=f"vps{i}")
                _c_chunk(cp, i % 4, vt0 if i < 4 else vt1, range(8))
                _c_evac(cp, i)

            _ar_consume(0)

            # D1: s-tiles 0-3 AV chains for ALL heads (independent of the
            # second Z allreduce) -- PE filler behind B1 in the queue.  The
            # two t-halves chain into single-bank tiles from two pools so
            # consecutive heads pipeline without waiting on the ACT evac.
            for h in range(HL):
                psA0 = dpsum.tile([P, 512], f32, tag="avp")
                psA1 = cpsum.tile([P, 512], f32, tag="vps")
                for i in range(AR_LO):
                    lw = vaug[i][:, h, :]
                    nc.tensor.matmul(
                        psA0[0:65, :], lw, wre[:, i, h, 0:512],
                        start=(i == 0), stop=(i == AR_LO - 1),
                    )
                    nc.tensor.matmul(
                        psA1[0:65, :], lw, wre[:, i, h, 512:1024],
                        start=(i == 0), stop=(i == AR_LO - 1),
                    )
                nc.scalar.copy(tmpA[h][0:65, 0, :], psA0[0:65, :])
                nc.scalar.copy(tmpA[h][0:65, 1, :], psA1[0:65, :])

            _ar_consume(1)

            # dummy-matmul bridge keeps HAM warm across the AR1 wait
            for _ in range(NBRIDGE):
                br = dpsum.tile([P, 512], f32, tag="avp")
                nc.tensor.matmul(
                    br[:], scratch[:, 0:128], scratch[:],
                    start=True, stop=True,
                )

        bstuff.close()   # ebt/stg/vt/wv space freed for the D/E pools

        # ---- phase D2: AV s-tiles 4-7 + grouped renorm ----
        dstack = ctx.enter_context(ExitStack())
        dpool = dstack.enter_context(tc.tile_pool(name="dpool", bufs=1))
        wo = dpool.tile([P, 4, D], f16)
        for ch in range(4):
            nc.sync.dma_start(wo[:, ch, :], wol[:, ch])
        # r-rows for all 8 heads: head h -> partition 32*(h%4), block h//4.
        # memset to 1 so Ln on unused rows is finite.  f16 + in-place
        # Ln/Exp; the renormed numerators overwrite tmpA in place, with
        # tmpA[2j] doubling as aop[j] for the E matmuls.
        rcol = dpool.tile([P, 2, T], f16)
        nc.gpsimd.memset(rcol[:], 1.0)
        aop = [tmpA[2 * j][:].rearrange("p a b -> p (a b)") for j in range(4)]

        d2stack = ctx.enter_context(ExitStack())
        d2psum = d2stack.enter_context(
            tc.tile_pool(name="d2psum", bufs=2, space="PSUM")
        )
        pbp = d2stack.enter_context(
            tc.tile_pool(name="pbp", bufs=1, space="PSUM")
        )
        for h in range(HL):
            psB = d2psum.tile([P, 2, 512], f32, tag="avp2")
            for i in range(5, 8):
                lw = vaug[i][:, h, :]
                nc.tensor.matmul(
                    psB[0:65, 0, :], lw, wre[:, i, h, 0:512],
                    start=(i == 5), stop=False,
                )
                nc.tensor.matmul(
                    psB[0:65, 1, :], lw, wre[:, i, h, 512:1024],
                    start=(i == 5), stop=False,
                )
            # merge the D1 partial via an identity matmul (PE adds are
            # cheap; DVE is the scarcer engine here), re-evacuate the summed
            # numerator on ACT, then scatter the r-row into rcol's
            # 32-aligned layout
            for th in range(2):
                nc.tensor.matmul(
                    psB[0:65, th, :], ident16[0:65, 0:65],
                    tmpA[h][0:65, th, :], start=False, stop=True,
                )
            nc.scalar.copy(tmpA[h][0:65, :, :], psB[0:65, :, :])
            rp = 32 * (h % 4)
            nc.vector.tensor_copy(
                rcol[rp:rp + 1, h // 4, :],
                tmpA[h][64:65, :, :].rearrange("p a b -> p (a b)"),
            )
            if h % 4 == 3:
                g = h // 4
                # 1/(r+eps) for this 4-head group in two ACT calls
                nc.scalar.activation(
                    rcol[:, g, :], rcol[:, g, :], AF.Ln, bias=eps_t[:]
                )
                nc.scalar.activation(
                    rcol[:, g, :], rcol[:, g, :], AF.Exp, scale=-1.0
                )
                for j in (2 * g, 2 * g + 1):
                    pb = pbp.tile([P, 2, 512], f32, tag="bcp")
                    for u in range(2):
                        hh = 2 * j + u
                        rp2 = 32 * (hh % 4)
                        for th in range(2):
                            nc.tensor.matmul(
                                pb[u * 64:(u + 1) * 64, th, :],
                                ones_bc[rp2:rp2 + 1, :],
                                rcol[rp2:rp2 + 1, g,
                                     th * 512:(th + 1) * 512],
                                start=True, stop=True,
                                tile_position=(rp2, u * 64),
                            )
                    # u=1 runs first: it reads tmpA[2j+1] and writes rows
                    # 64:128 of tmpA[2j] (aop[j]), so the u=0 in-place
                    # multiply on rows 0:64 never conflicts.
                    for u in (1, 0):
                        hh = 2 * j + u
                        nc.vector.tensor_mul(
                            out=aop[j][u * 64:(u + 1) * 64, :],
                            in0=tmpA[hh][0:64, :, :].rearrange(
                                "p a b -> p (a b)"),
                            in1=pb[u * 64:(u + 1) * 64, :, :].rearrange(
                                "p a b -> p (a b)"),
                        )
        d2stack.close()

        # ---- phase E: output projection (row-parallel partial; the pair
        # sum happens in the host-side unshard). ----
        estack = ctx.enter_context(ExitStack())
        epool = estack.enter_context(tc.tile_pool(name="epool", bufs=2))
        opsum = estack.enter_context(
            tc.tile_pool(name="opsum", bufs=2, space="PSUM")
        )
        for tch in range(8):
            pso = opsum.tile([P, 2, 512], f32, tag="outp")
            for j in range(4):
                lw = aop[j][:, tch * P:(tch + 1) * P]
                nc.tensor.matmul(
                    pso[:, 0, :], lw, wo[:, j, 0:512],
                    start=(j == 0), stop=(j == 3),
                )
                nc.tensor.matmul(
                    pso[:, 1, :], lw, wo[:, j, 512:1024],
                    start=(j == 0), stop=(j == 3),
                )
            ot = epool.tile([P, 2, 512], f16, tag="ot")
            nc.scalar.copy(ot[:], pso[:])
            nc.sync.dma_start(
                out[tch * P:(tch + 1) * P, :],
                ot[:].rearrange("p a b -> p (a b)"),
            )


def build_nc():
    nc = bass.Bass(num_devices=NCORES)
    qtl = nc.dram_tensor("qtl", [P, 8, T], f16, kind="ExternalInput")
    ktl = nc.dram_tensor("ktl", [P, 8, S], f16, kind="ExternalInput")
    vtl = nc.dram_tensor("vtl", [P, 8, S], f16, kind="ExternalInput")
    wql = nc.dram_tensor("wql", [P, 4, 8, P], f16, kind="ExternalInput")
    wkl = nc.dram_tensor("wkl", [P, 4, 8, P], f16, kind="ExternalInput")
    wvl = nc.dram_tensor("wvl", [P, 8, FL], f16, kind="ExternalInput")
    wol = nc.dram_tensor("wol", [P, 4, D], f16, kind="ExternalInput")
    biasl = nc.dram_tensor("biasl", [8, 4, P, 4, 512], fp8,
                           kind="ExternalInput")
    out = nc.dram_tensor("out", [T, D], f16, kind="ExternalOutput")
    with tile.TileContext(nc) as tc:
        _emit(nc, tc, qtl, ktl, vtl, wql, wkl, wvl, wol, biasl, out)
    _split_multi_waits(nc)
    return nc


_NC_CACHE = {}


def _get_nc():
    if "nc" not in _NC_CACHE:
        _NC_CACHE["nc"] = build_nc()
    return _NC_CACHE["nc"]


# ---------------------------------------------------------------------------
# Host wrapper
# ---------------------------------------------------------------------------
def _prep_inputs(q, k, v, attn_mask, attn_bias, Wq, Wk, Wv, Wo):
    q = np.asarray(q, np.float32)
    k = np.asarray(k, np.float32)
    v = np.asarray(v, np.float32)
    mask = np.asarray(attn_mask)
    bias = np.asarray(attn_bias, np.float32)
    WqT = np.asarray(Wq, np.float32).T * np.float32(DH ** -0.5)
    WkT = np.ascontiguousarray(np.asarray(Wk, np.float32).T)
    WvT = np.ascontiguousarray(np.asarray(Wv, np.float32).T)
    WoT = np.ascontiguousarray(np.asarray(Wo, np.float32).T)

    def _tile_qkv(x):     # [T, D] -> [p, c, t] f16
        return np.ascontiguousarray(
            x.T.reshape(8, P, T).transpose(1, 0, 2)).astype(np.float16)

    def _tile_wqk(wt, hh):   # [D, D] col-slice -> [p, f, o, g] f16
        sl = wt[:, hh * FL:(hh + 1) * FL]
        return np.ascontiguousarray(
            sl.reshape(8, P, 4, P).transpose(1, 2, 0, 3)).astype(np.float16)

    # bias + additive mask (exp underflows to 0 on device), per head-half,
    # tiled [i, f, p, th*2+u, t'] with h = 2f+u, s = i*128+p, t = th*512+t'
    pre = bias + np.where(mask, np.float32(NEG), np.float32(0.0))[None]
    w_hh = []
    for hh in range(2):
        ph = pre[hh * HL:(hh + 1) * HL]      # [8h, t, s]
        arr = ph.reshape(4, 2, 2, 512, 8, P).transpose(4, 0, 5, 2, 1, 3)
        arr = np.clip(arr, -240.0, 240.0)
        bias8 = np.ascontiguousarray(arr).reshape(8, 4, P, 4, 512) \
            .astype(ml_dtypes.float8_e4m3)
        w_hh.append({
            "wql": _tile_wqk(WqT, hh),
            "wkl": _tile_wqk(WkT, hh),
            "wvl": np.ascontiguousarray(
                WvT[:, hh * FL:(hh + 1) * FL].reshape(8, P, FL)
                .transpose(1, 0, 2)).astype(np.float16),
            "wol": np.ascontiguousarray(
                WoT[hh * FL:(hh + 1) * FL, :].reshape(4, P, D)
                .transpose(1, 0, 2)).astype(np.float16),
            "biasl": bias8,
        })

    in_maps = []
    for c in range(NCORES):
        b, hh = c // 2, c % 2
        if hh == 0:
            qt, kt, vt = _tile_qkv(q[b]), _tile_qkv(k[b]), _tile_qkv(v[b])
        in_maps.append({
            "qtl": qt, "ktl": kt, "vtl": vt, **w_hh[hh],
        })
    return in_maps


def run(inputs, trace=False):
    nc = _get_nc()
    in_maps = _prep_inputs(**inputs)
    res = run_bass_kernel_spmd(
        nc, in_maps, core_ids=list(range(NCORES)), trace=trace,
    )
    full = np.empty((B, T, D), np.float32)
    for b in range(B):
        full[b] = (
            np.asarray(res.results[2 * b]["out"], dtype=np.float32)
            + np.asarray(res.results[2 * b + 1]["out"], dtype=np.float32)
        )
    return full, res


def kernel(**inputs) -> np.ndarray:
    full, _ = run(inputs, trace=False)
    return full
